# revision 1
# baseline (speedup 1.0000x reference)
"""BCQ quantized linear (nn_BCQLinear) on 8 Trainium2 NeuronCores.

Computes: out[:, out_reorder] = x[:, in_reorder] @ dequant(qweight, alpha, beta,
block_bitwidth), where dequant is the BCQ bit-plane format with per-(group,
out-block) mixed bitwidths.

Strategy (hybrid tensor-parallel, 2-way batch x 4-way out_features):
  - Algebra: W[k,n] = alpha*(2*(v & m) - m) + beta with v = 4-bit code,
    m = 2^nb - 1  ==>  W = (2*alpha)*(v & m) + (beta - alpha*m).
  - Host staging (layout only + static weight-format repack):
      * bitplanes -> nibble codes v [IN_F, OUT_F] u8
      * out_reorder folded into per-core column selection (each core computes a
        contiguous TARGET column block; gathers its source columns)
      * in_reorder folded into a gathered+transposed activation copy
        xT[k, m] = x[m, in_reorder[k]]
  - Device (per core):
      * computes mask/scale tables from alpha/beta/bitwidth (DVE)
      * dequantizes its W shard [4096, 512..1024] (DVE) + PE-transposes into a
        resident fp32r weight matrix
      * streams x m-tiles, fp32r matmuls with fp32 PSUM accumulation
  - fp32r (TF32-like fast fp32 matmul mode): ~1.5e-4 relative error vs fp32,
    4x faster than exact-fp32 matmul.
"""
import numpy as np
from contextlib import ExitStack

import concourse.bass as bass  # noqa: F401  (engine namespaces live on nc)
import concourse.mybir as mybir
from concourse import bacc
from concourse.tile import TileContext
from concourse.bass_utils import run_bass_kernel_spmd

# ---- problem constants (hardcoded per contract) ----
BATCH = 8192
IN_F = 4096
OUT_F = 4096
GROUP = 128
INTERVAL = 32
NBITS = 4
G = IN_F // GROUP          # 32 k-groups
P = 128

MS, NS = 2, 4              # batch-shards x outfeature-shards (8 cores)
M_CORE = BATCH // MS
N_CORE = OUT_F // NS

f32 = mybir.dt.float32
f32r = mybir.dt.float32r
i32 = mybir.dt.int32
u8 = mybir.dt.uint8
A = mybir.AluOpType
AF = mybir.ActivationFunctionType


def _emit_core(nc, tc, ctx, m_core, n_core, xT, nibT, alphaT, betaT, nbT, ident, y,
               kc_chunk=2048):
    """Per-core kernel IR: coefficient tables -> dequant -> matmul m-loop."""
    MT = m_core // P
    NT = n_core // P
    NB = n_core // 512
    KCC = IN_F // kc_chunk
    GC = kc_chunk // P

    wpool = ctx.enter_context(tc.tile_pool(name="wres", bufs=1))
    cpool = ctx.enter_context(tc.tile_pool(name="coef", bufs=1))
    dqpool = ctx.enter_context(tc.tile_pool(name="dq", bufs=2))
    xpool = ctx.enter_context(tc.tile_pool(name="xs", bufs=2))
    opool = ctx.enter_context(tc.tile_pool(name="os", bufs=2))
    tps = ctx.enter_context(tc.tile_pool(name="tps", bufs=3, space="PSUM"))
    yps = ctx.enter_context(tc.tile_pool(name="yps", bufs=2, space="PSUM"))

    # Phase A: coefficient tables (mask, 2*alpha, beta-alpha*mask) on-device
    al_t = cpool.tile([P, NT, G], f32, tag="al")
    be_t = cpool.tile([P, NT, G], f32, tag="be")
    nb_t = cpool.tile([P, NT, G], i32, tag="nb")
    nc.sync.dma_start(al_t[:], alphaT.ap().rearrange("(nt p) g -> p nt g", p=P))
    nc.sync.dma_start(be_t[:], betaT.ap().rearrange("(nt p) g -> p nt g", p=P))
    nc.sync.dma_start(nb_t[:], nbT.ap().rearrange("(nt p) g -> p nt g", p=P))

    ones_t = cpool.tile([P, NT, G], i32, tag="ones")
    nc.vector.memset(ones_t[:], 1)
    mask_i = cpool.tile([P, NT, G], i32, tag="maski")
    nc.vector.tensor_tensor(mask_i[:], ones_t[:], nb_t[:], A.logical_shift_left)
    nc.vector.tensor_scalar(mask_i[:], mask_i[:], 1, None, A.subtract)
    mask_u = cpool.tile([P, NT, G], u8, tag="masku")
    nc.vector.tensor_copy(mask_u[:], mask_i[:])
    mask_f = cpool.tile([P, NT, G], f32, tag="maskf")
    nc.vector.tensor_copy(mask_f[:], mask_i[:])
    a2_t = cpool.tile([P, NT, G], f32, tag="a2")
    nc.vector.tensor_scalar(a2_t[:], al_t[:], 2.0, None, A.mult)
    c2_t = cpool.tile([P, NT, G], f32, tag="c2")
    nc.vector.tensor_tensor(c2_t[:], al_t[:], mask_f[:], A.mult)
    nc.vector.tensor_tensor(c2_t[:], be_t[:], c2_t[:], A.subtract)

    id_t = cpool.tile([P, P], f32r, tag="ident")
    nc.gpsimd.dma_start(id_t[:], ident.ap())  # f32 -> f32r cast

    # Phase B: dequant into resident W [P(k%128), G(k//128), n_core] f32r
    w_res = wpool.tile([P, G, n_core], f32r, tag="wres")
    for nt in range(NT):
        nib_t = dqpool.tile([P, IN_F], u8, tag="nib")
        nc.sync.dma_start(nib_t[:], nibT.ap()[nt * P:(nt + 1) * P, :])
        for kc in range(KCC):
            wt_t = dqpool.tile([P, kc_chunk], f32r, tag="wt")
            vm_t = dqpool.tile([P, kc_chunk], u8, tag="vm")
            for gc in range(GC):
                g = kc * GC + gc
                sl = slice(gc * P, (gc + 1) * P)
                nc.vector.tensor_scalar(
                    vm_t[:, sl], nib_t[:, g * P:(g + 1) * P],
                    mask_u[:, nt, g:g + 1], None, A.bitwise_and)
                nc.vector.tensor_scalar(
                    wt_t[:, sl], vm_t[:, sl],
                    a2_t[:, nt, g:g + 1], c2_t[:, nt, g:g + 1], A.mult, A.add)
            for gc in range(GC):
                g = kc * GC + gc
                pt = tps.tile([P, P], f32r, tag="tp")
                nc.tensor.transpose(pt[:], wt_t[:, gc * P:(gc + 1) * P], id_t[:])
                nc.scalar.activation(w_res[:, g, nt * P:(nt + 1) * P], pt[:], AF.Copy)

    # Phase C: matmul m-loop
    for mt in range(MT):
        x_t = xpool.tile([P, G, P], f32r, tag="x")
        nc.gpsimd.dma_start(
            x_t[:],
            xT.ap().rearrange("(kt p) m -> p kt m", p=P)[:, :, mt * P:(mt + 1) * P])
        o_t = opool.tile([P, n_core], f32, tag="o")
        for nb in range(NB):
            pt = yps.tile([P, 512], f32, tag=f"ypsum{nb % 2}")
            for kt in range(G):
                nc.tensor.matmul(
                    pt[:], x_t[:, kt, :], w_res[:, kt, nb * 512:(nb + 1) * 512],
                    start=(kt == 0), stop=(kt == G - 1))
            nc.scalar.activation(o_t[:, nb * 512:(nb + 1) * 512], pt[:], AF.Copy)
        nc.sync.dma_start(y.ap()[mt * P:(mt + 1) * P, :], o_t[:])


def _build_module():
    nc = bacc.Bacc("TRN2", target_bir_lowering=False, debug=False, num_devices=1)
    xT = nc.dram_tensor("xT", [IN_F, M_CORE], f32, kind="ExternalInput")
    nibT = nc.dram_tensor("nibT", [N_CORE, IN_F], u8, kind="ExternalInput")
    alphaT = nc.dram_tensor("alphaT", [N_CORE, G], f32, kind="ExternalInput")
    betaT = nc.dram_tensor("betaT", [N_CORE, G], f32, kind="ExternalInput")
    nbT = nc.dram_tensor("nbT", [N_CORE, G], i32, kind="ExternalInput")
    ident = nc.dram_tensor("ident", [P, P], f32, kind="ExternalInput")
    y = nc.dram_tensor("y", [M_CORE, N_CORE], f32, kind="ExternalOutput")
    with TileContext(nc) as tc:
        with ExitStack() as ctx:
            _emit_core(nc, tc, ctx, M_CORE, N_CORE,
                       xT, nibT, alphaT, betaT, nbT, ident, y)
    nc.compile()
    return nc


def _pack_nibbles(qweight):
    """bitplanes [IN_F//32, NBITS, OUT_F] u32 -> nibble codes [IN_F, OUT_F] u8."""
    q = np.asarray(qweight)
    shifts = np.arange(32, dtype=np.uint32)
    bits = ((q[:, None, :, :] >> shifts[None, :, None, None]) & np.uint32(1)).astype(np.uint8)
    bits = bits.reshape(IN_F, NBITS, OUT_F)
    weights = (np.uint8(1) << np.arange(NBITS, dtype=np.uint8))[None, :, None]
    return (bits * weights).sum(axis=1).astype(np.uint8)


def _shard_inputs(x, qweight, alpha, beta, in_reorder, out_reorder, block_bitwidth):
    x = np.ascontiguousarray(np.asarray(x), dtype=np.float32)
    alpha = np.asarray(alpha)
    beta = np.asarray(beta)
    in_reorder = np.asarray(in_reorder)
    out_reorder = np.asarray(out_reorder)
    block_bitwidth = np.asarray(block_bitwidth)

    inv = np.empty(OUT_F, dtype=np.int64)
    inv[out_reorder] = np.arange(OUT_F)
    nib = _pack_nibbles(qweight)
    ident = np.eye(P, dtype=np.float32)
    xg_T = np.ascontiguousarray(x[:, in_reorder].T)

    in_maps = []
    for c in range(MS * NS):
        im, inn = divmod(c, NS)
        jl = inv[inn * N_CORE:(inn + 1) * N_CORE]
        in_maps.append(dict(
            xT=np.ascontiguousarray(xg_T[:, im * M_CORE:(im + 1) * M_CORE]),
            nibT=np.ascontiguousarray(nib[:, jl].T),
            alphaT=np.ascontiguousarray(alpha[:, jl].T),
            betaT=np.ascontiguousarray(beta[:, jl].T),
            nbT=np.ascontiguousarray(block_bitwidth[:, jl // INTERVAL].T.astype(np.int32)),
            ident=ident,
        ))
    return in_maps


_NC = None


def kernel(x, qweight, alpha, beta, in_reorder, out_reorder, block_bitwidth):
    global _NC
    if _NC is None:
        _NC = _build_module()
    in_maps = _shard_inputs(x, qweight, alpha, beta,
                            in_reorder, out_reorder, block_bitwidth)
    res = run_bass_kernel_spmd(_NC, in_maps, core_ids=list(range(MS * NS)))
    out = np.empty((BATCH, OUT_F), dtype=np.float32)
    for c, r in enumerate(res.results):
        im, inn = divmod(c, NS)
        out[im * M_CORE:(im + 1) * M_CORE, inn * N_CORE:(inn + 1) * N_CORE] = r["y"]
    return out
